# revision 42
# baseline (speedup 1.0000x reference)
"""Trainium2 Bass kernel for nn_NisuyNN_90434831384984.

Math: the reference's stack+reshape makes MLP row (s,t,b) depend only on s
(b in {0,1}) or only on t (b in {2,3}) -- 64 unique rows through the MLP
produce 64 unique 32x32 policy matrices.  The reference applies LeakyReLU
to ALL six layers (including layer 6) before the sigmoid.

Final layout (194.8us HW, vs 257.2us measured baseline):
  - L1 is computed IN FULL on every core in transposed orientation
    (lhsT=W1 tiles, rhs=X^T; per-partition bias rides the Lrelu
    activation), so its activations land directly in next-layer lhsT
    layout with NO AllGather -- the whole layer plus L2's matmuls hide
    inside the collectives-init barrier window.  L2..L4: Megatron
    column-split (512 cols/core); AllGather of the transposed fp8
    activations after each layer; the 32 K-chunks pack the PE as two
    concurrent M=64 column groups.
  - W2..W5 are fp8(e4m3) scaled x64 (the /64 rides the activation's
    scale operand, exactly), halving the HBM weight stream so it drains
    inside the ~50us collectives-init barrier window and never contends
    with the latency-critical gather DMAs.
  - L5: column-split, no gather.  L6: row-split against the core's W6
    row-slice (W6 columns host-permuted so output rows are M^T); one bf16
    ReduceScatter sums the partials AND shards the 64 rows 8-per-core.
  - Tail: bias+LeakyReLU+sigmoid+scale into two 128x128 block-diagonal
    bf16 matrices X=diag(M_r^T); the 8-step power iteration becomes
    2 PE squarings (M^4 is converged; X_{k+1}=Y_k^T X_k with
    Y=StreamTranspose(X); scale cancels in the delta ratios); bv is
    extracted straight into an [8, 32] rows-on-partitions layout by two
    accumulated selection matmuls (SEL8A/SEL8B) plus one strided reduce,
    so the delta-coefficient tail runs 8-wide -- no PE transposes, no
    DRAM round trip, no slow single-lane reciprocals.  Final tiny
    AllGather of per-core [4, 32] partial deltas; every core sums the
    8 blocks.
  - Dummy warm-spin matmuls after each layer keep the PE HAM clock at
    2.4 GHz across the collective windows; Lrelu/Sigmoid ACT tables are
    preloaded at startup.
"""

import numpy as np

DIM = 128
N = 32
B = 4
H = 4096
NC = 8          # cores
SL = H // NC    # 512 hidden slice per core
OF = N * N      # 1024 output features
R = 64          # unique MLP rows
HR = 32         # rows per stream
KC = 128        # contraction chunk
SLOPE = 0.01
SC = 64.0       # fp8 weight scale (power of two; exact)
WSPIN = 48      # dummy warm matmuls spanning each AG window

_COMPILED = None
LAST_RESULTS = None


def _build_body(nc, tc, tile, mybir, aps):
    f32 = mybir.dt.float32
    bf16 = mybir.dt.bfloat16
    f8 = mybir.dt.float8e4
    AF = mybir.ActivationFunctionType
    ALU = mybir.AluOpType
    AX = mybir.AxisListType
    rg = [list(range(NC))]

    from contextlib import ExitStack
    es = ExitStack()
    cpool = es.enter_context(tc.tile_pool(name="consts", bufs=1))
    wpool = es.enter_context(tc.tile_pool(name="w", bufs=1))
    bpool = es.enter_context(tc.tile_pool(name="b", bufs=1))
    apool = es.enter_context(tc.tile_pool(name="act", bufs=2))
    atp = es.enter_context(tc.tile_pool(name="atT", bufs=2))
    lpool = es.enter_context(tc.tile_pool(name="lhs", bufs=2))
    tailp = es.enter_context(tc.tile_pool(name="tail", bufs=1))
    ps = es.enter_context(tc.tile_pool(name="ps", bufs=2, space="PSUM"))
    pst = es.enter_context(tc.tile_pool(name="pst", bufs=2, space="PSUM"))
    ps6 = es.enter_context(tc.tile_pool(name="ps6", bufs=1, space="PSUM"))
    tps = es.enter_context(tc.tile_pool(name="tps", bufs=2, space="PSUM"))
    dram = es.enter_context(tc.tile_pool(name="dram", bufs=1, space="DRAM"))

    # ---- preload the Lrelu/Sigmoid activation tables during startup ----
    scr0 = cpool.tile([1, 2], f32)
    nc.vector.memset(scr0[:], 0.0)
    scr1 = cpool.tile([1, 2], f32)
    nc.scalar.activation(scr1[:], scr0[:], AF.Lrelu, alpha=SLOPE)
    scr2 = cpool.tile([1, 2], f32)
    nc.scalar.activation(scr2[:], scr0[:], AF.Sigmoid)

    # ---- all input DMAs up front, in consumption order ----
    id64 = cpool.tile([64, 64], bf16)
    nc.sync.dma_start(id64[:], aps["ID64"][:])
    xt = wpool.tile([KC, 2 * R], bf16, tag="xt")
    nc.sync.dma_start(xt[:], aps["XT"][:])
    wts = {}
    bts = {}
    w1f = wpool.tile([KC, 2 * H], bf16, tag="w1f")
    nc.sync.dma_start(w1f[:], aps["W1F"][:])
    b1f = bpool.tile([KC, H // KC], f32, tag="b1f")
    nc.sync.dma_start(b1f[:], aps["B1F"][:])
    for li in range(2, 7):
        nk = H // KC if li < 6 else 4
        width = OF if li == 6 else SL
        dt = f8 if li <= 5 else bf16
        wts[li] = wpool.tile([KC, nk * width], dt, tag=f"w{li}",
                             name=f"wt{li}")
        nc.sync.dma_start(wts[li][:], aps[f"W{li}"][:])
        if li < 6:
            bts[li] = bpool.tile([1, SL], bf16, tag=f"b{li}", name=f"bt{li}")
            nc.sync.dma_start(bts[li][0:1, :], aps[f"b{li}"].unsqueeze(0))

    # ---- constants (scalar queue; gpsimd stays free for CC doorbells) ----
    bias6 = cpool.tile([128, N], f32)
    nc.scalar.dma_start(bias6[:], aps["BIAS6"][:])
    mac = cpool.tile([128, 2], f32)
    nc.scalar.dma_start(mac[:], aps["MAC"][:])
    dm8 = cpool.tile([8, N], f32)
    nc.scalar.dma_start(dm8[:], aps["DM8"][:])
    tt8 = cpool.tile([8, N], f32)
    nc.scalar.dma_start(tt8[:], aps["TT8"][:])
    w01c = cpool.tile([8, 1], f32)
    nc.scalar.dma_start(w01c[:], aps["W01C"][:])
    sels = cpool.tile([8, B], f32)
    nc.scalar.dma_start(sels[:], aps["SELS"][:])
    selt = cpool.tile([8, B], f32)
    nc.scalar.dma_start(selt[:], aps["SELT"][:])
    sel8a = cpool.tile([128, 8], bf16)
    nc.scalar.dma_start(sel8a[:], aps["SEL8A"][:])
    sel8b = cpool.tile([128, 8], bf16)
    nc.scalar.dma_start(sel8b[:], aps["SEL8B"][:])
    onesb = cpool.tile([1, R], bf16)
    nc.vector.memset(onesb[:], 1.0)
    x1a = tailp.tile([128, 128], bf16, tag="x1a")
    nc.vector.memset(x1a[:], 0.0)
    x1b = tailp.tile([128, 128], bf16, tag="x1b")
    nc.vector.memset(x1b[:], 0.0)

    def layer_mm(chunks, wt, btile, li):
        """chunks: list of (lhs_ap, global_k).  Returns the [128, SL] psum
        with the two 64-wide column-group partial sums in rows [0:64] and
        [64:128]; bias is accumulated into group 0."""
        pt = ps.tile([2 * R, SL], f32, tag="ps", name=f"pt{li}")
        first = [True, True]
        n_h1 = sum(1 for i in range(len(chunks)) if i % 2 == 1)
        seen_h1 = 0
        for i, (lhs, k) in enumerate(chunks):
            h = i % 2
            if h == 1:
                seen_h1 += 1
            nc.tensor.matmul(
                pt[h * R:(h + 1) * R, :],
                lhs,
                wt[:, k * SL:(k + 1) * SL],
                start=first[h],
                stop=(h == 1 and seen_h1 == n_h1),
                tile_position=(0, h * R),
                skip_group_check=True,
            )
            first[h] = False
        nc.tensor.matmul(
            pt[0:R, :], onesb[0:1, :], btile[0:1, :],
            start=first[0], stop=True, tile_position=(0, 0),
            skip_group_check=True,
        )
        return pt

    def act_transpose(pt, li, scale):
        """psum halves -> z -> LeakyReLU (fp8 unscale folded into the
        activation scale) -> bf16 -> transposed fp8 att tile."""
        z0 = apool.tile([R, SL], f32, tag="z0", name=f"z0{li}")
        nc.scalar.activation(z0[:], pt[0:R, :], AF.Copy)
        z = apool.tile([R, SL], f32, tag="z", name=f"z{li}")
        nc.vector.tensor_tensor(z[:], z0[:], pt[R:2 * R, :], op=ALU.add)
        act = apool.tile([R, SL], bf16, tag="act", name=f"act{li}")
        nc.scalar.activation(act[:], z[:], AF.Lrelu, alpha=SLOPE, scale=scale)
        att = atp.tile([KC, 4 * R], f8, tag="att", name=f"att{li}")
        ag_in = dram.tile([KC, 4 * R], f8, tag=f"agin{li}", name=f"agin{li}")
        for j in range(4):
            tp = pst.tile([KC, R], bf16, tag="pst", name=f"tp{li}_{j}")
            nc.tensor.transpose(tp[:], act[:, j * KC:(j + 1) * KC], id64[:])
            nc.vector.tensor_copy(att[:, j * R:(j + 1) * R], tp[:])
            if j % 2 == 1:
                eng = nc.scalar if j == 1 else nc.sync
                eng.dma_start(ag_in[:, (j - 1) * R:(j + 1) * R],
                              att[:, (j - 1) * R:(j + 1) * R])
        return att, ag_in

    def gather(ag_in, li):
        ag_out = dram.tile([NC * KC, 4 * R], f8, tag=f"agout{li}",
                           addr_space="Shared")
        nc.gpsimd.collective_compute(
            "AllGather", ALU.bypass, replica_groups=rg,
            ins=[ag_in[:].opt()], outs=[ag_out[:].opt()],
        )
        engs = [nc.sync, nc.scalar, nc.gpsimd]
        chunks = []
        for r in range(NC):
            lt = lpool.tile([KC, 4 * R], f8, tag=f"lt{r}", name=f"lt{li}_{r}")
            engs[r % 3].dma_start(lt[:], ag_out[r * KC:(r + 1) * KC, :])
            for j in range(4):
                chunks.append((lt[:, j * R:(j + 1) * R], r * 4 + j))
        return chunks

    def warm_spin(count, li, fine=24):
        for i in range(count):
            dpt = ps.tile([2 * R, SL], f32, tag="ps", name=f"wsp{li}_{i}")
            nc.tensor.matmul(dpt[0:R, :], xt[:, 0:R], wts[2][:, 0:SL],
                             start=True, stop=True, tile_position=(0, 0),
                             skip_group_check=True)
        for i in range(fine):
            dpt = ps.tile([2 * R, SL], f32, tag="ps", name=f"wsf{li}_{i}")
            nc.tensor.matmul(dpt[0:R, 0:64], xt[:, 0:R], wts[2][:, 0:64],
                             start=True, stop=True, tile_position=(0, 0),
                             skip_group_check=True)

    # ---- L1: full transposed layer on every core (hidden entirely under
    # the collectives-init barrier; eliminates the first AllGather) ----
    att1 = atp.tile([KC, (H // KC) * R], f8, tag="att1", bufs=1)
    for mt in range(H // KC):
        pc = pst.tile([KC, R], f32, tag="pst", name=f"l1c{mt}")
        nc.tensor.matmul(pc[:], w1f[:, mt * KC:(mt + 1) * KC],
                         xt[:, 0:R], start=True, stop=False,
                         tile_position=(0, 0), skip_group_check=True)
        nc.tensor.matmul(pc[:], w1f[:, H + mt * KC:H + (mt + 1) * KC],
                         xt[:, R:2 * R], start=False, stop=True,
                         tile_position=(0, 0), skip_group_check=True)
        nc.scalar.activation(att1[:, mt * R:(mt + 1) * R], pc[:],
                             AF.Lrelu, alpha=SLOPE,
                             bias=b1f[:, mt:mt + 1])
    chunks = [(att1[:, k * R:(k + 1) * R], k) for k in range(H // KC)]

    # ---- L2..L4 ----
    for li in range(2, 5):
        pt = layer_mm(chunks, wts[li], bts[li], li)
        att, ag_in = act_transpose(pt, li, 1.0 / SC)
        chunks = gather(ag_in, li)
        warm_spin(WSPIN, li)

    # ---- L5 (no gather) + L6 row-split partial, interleaved ----
    pt5 = ps.tile([2 * R, SL], f32, tag="ps", name="pt5")
    chunks5 = chunks
    pt = layer_mm(chunks5, wts[5], bts[5], 5) if False else None
    # inline L5 matmuls
    first = [True, True]
    seen_h1 = 0
    n_h1 = sum(1 for i in range(len(chunks5)) if i % 2 == 1)
    for i, (lhs, k) in enumerate(chunks5):
        h = i % 2
        if h == 1:
            seen_h1 += 1
        nc.tensor.matmul(pt5[h * R:(h + 1) * R, :], lhs,
                         wts[5][:, k * SL:(k + 1) * SL],
                         start=first[h], stop=(h == 1 and seen_h1 == n_h1),
                         tile_position=(0, h * R), skip_group_check=True)
        first[h] = False
    nc.tensor.matmul(pt5[0:R, :], onesb[0:1, :], bts[5][0:1, :],
                     start=first[0], stop=True, tile_position=(0, 0),
                     skip_group_check=True)
    z05 = apool.tile([R, SL], f32, tag="z0", name="z05")
    nc.scalar.activation(z05[:], pt5[0:R, :], AF.Copy)
    z5 = apool.tile([R, SL], f32, tag="z", name="z5")
    nc.vector.tensor_tensor(z5[:], z05[:], pt5[R:2 * R, :], op=ALU.add)
    act5 = apool.tile([R, SL], bf16, tag="act", name="act5")
    nc.scalar.activation(act5[:], z5[:], AF.Lrelu, alpha=SLOPE, scale=1.0 / SC)
    scr3 = cpool.tile([1, 2], f32)
    nc.scalar.activation(scr3[:], scr0[:], AF.Sigmoid)
    att5 = atp.tile([KC, 4 * R], f8, tag="att", name="att5")
    pt6 = ps6.tile([2 * R, SL], f32, tag="p6")
    for kc in range(4):
        tp = pst.tile([KC, R], bf16, tag="pst", name=f"tp5_{kc}")
        nc.tensor.transpose(tp[:], act5[:, kc * KC:(kc + 1) * KC], id64[:])
        nc.vector.tensor_copy(att5[:, kc * R:(kc + 1) * R], tp[:])
        lhs = att5[:, kc * R:(kc + 1) * R]
        nc.tensor.matmul(pt6[0:R, :], lhs, wts[6][:, kc * OF:kc * OF + SL],
                         start=(kc == 0), stop=(kc == 3),
                         tile_position=(0, 0), skip_group_check=True)
        nc.tensor.matmul(pt6[R:2 * R, :], lhs,
                         wts[6][:, kc * OF + SL:(kc + 1) * OF],
                         start=(kc == 0), stop=(kc == 3),
                         tile_position=(0, R), skip_group_check=True)
    z6 = apool.tile([R, OF], bf16, tag="z6", bufs=1)
    nc.vector.tensor_copy(z6[:, 0:SL], pt6[0:R, :])
    nc.scalar.activation(z6[:, SL:OF], pt6[R:2 * R, :], AF.Copy)
    rs_in = dram.tile([R, OF], bf16, tag="rsin")
    nc.sync.dma_start(rs_in[:], z6[:])
    rs_out = dram.tile([NC, OF], bf16, tag="rsout")
    nc.gpsimd.collective_compute(
        "ReduceScatter", ALU.add, replica_groups=rg,
        ins=[rs_in[:].opt()], outs=[rs_out[:].opt()],
    )

    # ---- tail: 8 rows on this core ----
    zza = tailp.tile([128, N], bf16, tag="zza")
    zzb = tailp.tile([128, N], bf16, tag="zzb")
    nc.sync.dma_start(
        zza[:], rs_out[0:4, :].rearrange("r (j i) -> (r j) i", i=N))
    nc.scalar.dma_start(
        zzb[:], rs_out[4:8, :].rearrange("r (j i) -> (r j) i", i=N))

    def poltile(zz, name, ve):
        zb = tailp.tile([128, N], f32, tag=f"zb_{name}")
        ve.tensor_tensor(zb[:], zz[:], bias6[:], op=ALU.add)
        sc = tailp.tile([128, N], f32, tag=f"sc_{name}")
        ve.tensor_scalar_mul(sc[:], zb[:], SLOPE)
        lr = tailp.tile([128, N], f32, tag=f"lr_{name}")
        ve.tensor_tensor(lr[:], zb[:], sc[:], op=ALU.max)
        sg = tailp.tile([128, N], f32, tag=f"sg_{name}")
        nc.scalar.activation(sg[:], lr[:], AF.Sigmoid)
        pol = tailp.tile([128, N], bf16, tag=f"pol_{name}")
        ve.tensor_scalar(pol[:], sg[:], mac[:, 0:1], mac[:, 1:2],
                         op0=ALU.mult, op1=ALU.add)
        return pol

    pola = poltile(zza, "a", nc.vector)
    polb = poltile(zzb, "b", nc.vector)
    for rl in range(4):
        s = slice(rl * 32, (rl + 1) * 32)
        nc.vector.tensor_copy(x1a[s, s], pola[s, :])
        nc.scalar.activation(x1b[s, s], polb[s, :], AF.Copy)

    def streamT(x, name):
        y = tailp.tile([128, 128], bf16, tag=f"y_{name}")
        nc.vector.transpose(y[:], x[:])
        return y

    y1a = streamT(x1a, "1a")
    y1b = streamT(x1b, "1b")

    def sq(x, y, name, want_y=True):
        px = tps.tile([128, 128], f32, tag="tps", name=f"px{name}")
        nc.tensor.matmul(px[:], y[:], x[:], start=True, stop=True)
        x2 = tailp.tile([128, 128], bf16, tag=f"x_{name}")
        nc.vector.tensor_copy(x2[:], px[:])
        if not want_y:
            return x2, None
        py = tps.tile([128, 128], f32, tag="tps", name=f"py{name}")
        nc.tensor.matmul(py[:], x[:], y[:], start=True, stop=True)
        y2 = tailp.tile([128, 128], bf16, tag=f"y_{name}")
        nc.scalar.activation(y2[:], py[:], AF.Copy)
        return x2, y2

    x2a, y2a = sq(x1a, y1a, "2a")
    x2b, y2b = sq(x1b, y1b, "2b")
    x8a, _ = sq(x2a, y2a, "4a", want_y=False)
    x8b, _ = sq(x2b, y2b, "4b", want_y=False)

    # per-block column sums of X8 via selection matmuls accumulated into
    # one [8, 128] psum (SEL8A maps x8a's 4 blocks to rows 0-3, SEL8B maps
    # x8b's to rows 4-7); one strided reduce densifies to [8, 32].
    bv_ps = tps.tile([8, 128], f32, tag="tps", name="bvps")
    nc.tensor.matmul(bv_ps[:], sel8a[:], x8a[:], start=True, stop=False)
    nc.tensor.matmul(bv_ps[:], sel8b[:], x8b[:], start=False, stop=True)
    bvs = tailp.tile([8, 128], f32, tag="bvs")
    nc.vector.tensor_copy(bvs[:], bv_ps[:])
    bv8 = tailp.tile([8, N], f32, tag="bv8")
    nc.vector.reduce_sum(
        bv8[:], bvs[:].rearrange("p (q j) -> p j q", j=N), axis=AX.X)

    # delta coefficients on 8 partitions
    recipE = tailp.tile([8, N], f32, tag="recipE")
    nc.vector.reciprocal(recipE[:], bv8[:])
    tmp = tailp.tile([8, N], f32, tag="tmp")
    nc.vector.tensor_tensor(tmp[:], bv8[:], dm8[:], op=ALU.mult)
    srcv = tailp.tile([8, 1], f32, tag="srcv")
    nc.vector.reduce_sum(srcv[:], tmp[:], axis=AX.X)
    rd = tailp.tile([8, 1], f32, tag="rd")
    nc.vector.reciprocal(rd[:], srcv[:])
    coefS = tailp.tile([8, 1], f32, tag="coefS")
    nc.vector.tensor_tensor(coefS[:], w01c[:], rd[:], op=ALU.mult)
    tmp2 = tailp.tile([8, N], f32, tag="tmp2")
    nc.vector.tensor_tensor(tmp2[:], tt8[:], recipE[:], op=ALU.mult)
    c23 = tailp.tile([8, 1], f32, tag="c23")
    nc.vector.reduce_sum(c23[:], tmp2[:], axis=AX.X)
    t3 = tailp.tile([8, B], f32, tag="t3")
    nc.vector.tensor_scalar_mul(t3[:], sels[:], coefS[:, 0:1])
    t4 = tailp.tile([8, B], f32, tag="t4")
    nc.vector.tensor_scalar_mul(t4[:], selt[:], c23[:, 0:1])
    coefL = tailp.tile([8, B], f32, tag="coefL")
    nc.vector.tensor_tensor(coefL[:], t3[:], t4[:], op=ALU.add)
    pd_ps = tps.tile([B, N], f32, tag="tps", name="pdps")
    nc.tensor.matmul(pd_ps[:], coefL[:], bv8[:], start=True, stop=True)
    pd = tailp.tile([B, N], f32, tag="pd")
    nc.vector.tensor_copy(pd[:], pd_ps[:])

    # final gather of per-core partial deltas + sum on every core
    agf_in = dram.tile([B, N], f32, tag="agfin")
    nc.scalar.dma_start(agf_in[:], pd[:])
    agf_out = dram.tile([NC * B, N], f32, tag="agfout", addr_space="Shared")
    nc.gpsimd.collective_compute(
        "AllGather", ALU.bypass, replica_groups=rg,
        ins=[agf_in[:].opt()], outs=[agf_out[:].opt()],
    )
    pdall = tailp.tile([B, NC * N], f32, tag="pdall")
    nc.scalar.dma_start(
        pdall[:].rearrange("b (k j) -> b k j", j=N),
        agf_out[:].rearrange("(k b) j -> b k j", b=B),
    )
    osb = tailp.tile([B, N], f32, tag="osb")
    nc.vector.reduce_sum(
        osb[:], pdall[:].rearrange("b (k j) -> b j k", j=N), axis=AX.X)
    nc.sync.dma_start(aps["out"][:], osb[:])
    es.close()


def build():
    import concourse.bacc as bacc
    import concourse.mybir as mybir
    import concourse.tile as tile

    f32 = mybir.dt.float32
    bf16 = mybir.dt.bfloat16
    f8 = mybir.dt.float8e4
    nc = bacc.Bacc("TRN2", target_bir_lowering=False, debug=False, num_devices=NC)
    shapes = {
        "XT": ([KC, 2 * R], bf16),
        "W1F": ([KC, 2 * H], bf16), "B1F": ([KC, H // KC], f32),
        "W2": ([KC, 32 * SL], f8), "b2": ([SL], bf16),
        "W3": ([KC, 32 * SL], f8), "b3": ([SL], bf16),
        "W4": ([KC, 32 * SL], f8), "b4": ([SL], bf16),
        "W5": ([KC, 32 * SL], f8), "b5": ([SL], bf16),
        "W6": ([KC, 4 * OF], bf16),
        "BIAS6": ([128, N], f32), "MAC": ([128, 2], f32),
        "DM8": ([8, N], f32), "TT8": ([8, N], f32), "W01C": ([8, 1], f32),
        "SELS": ([8, B], f32), "SELT": ([8, B], f32),
        "SEL8A": ([128, 8], bf16), "SEL8B": ([128, 8], bf16),
        "ID64": ([64, 64], bf16),
    }
    aps = {
        k: nc.dram_tensor(k, v[0], v[1], kind="ExternalInput").ap()
        for k, v in shapes.items()
    }
    aps["out"] = nc.dram_tensor("out", [B, N], f32, kind="ExternalOutput").ap()
    with tile.TileContext(nc) as tc:
        _build_body(nc, tc, tile, mybir, aps)
    nc.compile()
    return nc


def prep_in_maps(inputs):
    import ml_dtypes
    f = np.float32
    bf = ml_dtypes.bfloat16
    f8 = ml_dtypes.float8_e4m3fn
    E = np.asarray(inputs["batch_node_embeddings"], f)   # (B,N,D)
    T = np.asarray(inputs["batch_Ts"], f)                # (B,N,N)
    mult = np.asarray(inputs["mult_const_batch"], f).reshape(-1)[0]
    add = np.asarray(inputs["add_const_batch"], f).reshape(-1)[0]
    S = np.transpose(E, (1, 0, 2))                       # (N,B,D)
    G0 = np.concatenate([S[:, 0], S[:, 1]], axis=-1)     # (32, 2D)
    G1 = np.concatenate([S[:, 2], S[:, 3]], axis=-1)
    rows = np.concatenate([G0, G1], axis=0)              # (64, 256)

    def packk(Wslice):
        nk = Wslice.shape[0] // KC
        return np.ascontiguousarray(
            Wslice.reshape(nk, KC, -1).transpose(1, 0, 2).reshape(KC, -1)
        )

    perm = np.arange(OF).reshape(N, N).T.reshape(-1)     # perm[j*32+i] = i*32+j
    W6perm = np.asarray(inputs["W6"], f)[:, perm]
    b6p = np.asarray(inputs["b6"], f)[perm]

    common = {
        "XT": packk(rows.T).astype(bf),
        "BIAS6": np.ascontiguousarray(np.tile(b6p.reshape(N, N), (4, 1))),
        "MAC": np.ascontiguousarray(
            np.stack([np.full(128, mult, f), np.full(128, add, f)], axis=1)
        ),
        "ID64": np.eye(64, dtype=bf),
        "SEL8A": np.hstack([
            np.kron(np.eye(4, dtype=f), np.ones((N, 1), f)),
            np.zeros((128, 4), f)]).astype(bf),
        "SEL8B": np.hstack([
            np.zeros((128, 4), f),
            np.kron(np.eye(4, dtype=f), np.ones((N, 1), f))]).astype(bf),
    }
    W1 = np.asarray(inputs["W1"], f)
    b1 = np.asarray(inputs["b1"], f)
    # W1F[p, kc*H + mt*128 + m] = W1[kc*128+p, mt*128+m]
    common["W1F"] = np.ascontiguousarray(
        W1.reshape(2, KC, H).transpose(1, 0, 2).reshape(KC, 2 * H)
    ).astype(bf)
    common["B1F"] = np.ascontiguousarray(
        b1.reshape(H // KC, KC).T.astype(f))
    in_maps = []
    for c in range(NC):
        m = dict(common)
        for li in range(2, 6):
            W = np.asarray(inputs[f"W{li}"], f)
            b = np.asarray(inputs[f"b{li}"], f)
            m[f"W{li}"] = (packk(W[:, c * SL:(c + 1) * SL]) * SC).astype(f8)
            m[f"b{li}"] = np.ascontiguousarray(
                b[c * SL:(c + 1) * SL] * SC).astype(bf)
        m["W6"] = packk(W6perm[c * SL:(c + 1) * SL, :]).astype(bf)
        bS = 0 if c < 4 else 1
        bT = 2 if c < 4 else 3
        dm8 = np.zeros((8, N), f)
        tt8 = np.zeros((8, N), f)
        w01c = np.zeros((8, 1), f)
        sels = np.zeros((8, B), f)
        selt = np.zeros((8, B), f)
        for rl in range(8):
            s = (8 * c + rl) % N
            dm8[rl, s] = 1.0
            tt8[rl] = T[bT][:, s]
            w01c[rl, 0] = T[bS][s, :].sum()
            sels[rl, bS] = 1.0
            selt[rl, bT] = 1.0
        m["DM8"] = dm8
        m["TT8"] = tt8
        m["W01C"] = w01c
        m["SELS"] = sels
        m["SELT"] = selt
        in_maps.append(m)
    return in_maps


def kernel(**inputs):
    global _COMPILED, LAST_RESULTS
    from concourse import bass_utils

    if _COMPILED is None:
        _COMPILED = build()
    in_maps = prep_in_maps(inputs)
    res = bass_utils.run_bass_kernel_spmd(
        _COMPILED, in_maps, core_ids=list(range(NC))
    )
    LAST_RESULTS = res
    return np.asarray(res.results[0]["out"], np.float32)


# revision 43
# speedup vs baseline: 1.0439x; 1.0439x over previous
"""Trainium2 Bass kernel for nn_NisuyNN_90434831384984.

Math: the reference's stack+reshape makes MLP row (s,t,b) depend only on s
(b in {0,1}) or only on t (b in {2,3}) -- 64 unique rows through the MLP
produce 64 unique 32x32 policy matrices.  The reference applies LeakyReLU
to ALL six layers (including layer 6) before the sigmoid.

Final layout (194.8us HW, vs 257.2us measured baseline):
  - L1 is computed IN FULL on every core in transposed orientation
    (lhsT=W1 tiles, rhs=X^T; per-partition bias rides the Lrelu
    activation), so its activations land directly in next-layer lhsT
    layout with NO AllGather -- the whole layer plus L2's matmuls hide
    inside the collectives-init barrier window.  L2..L4: Megatron
    column-split (512 cols/core); AllGather of the transposed fp8
    activations after each layer; the 32 K-chunks pack the PE as two
    concurrent M=64 column groups.
  - W2..W5 are fp8(e4m3) scaled x64 (the /64 rides the activation's
    scale operand, exactly), halving the HBM weight stream so it drains
    inside the ~50us collectives-init barrier window and never contends
    with the latency-critical gather DMAs.
  - L5: column-split, no gather.  L6: row-split against the core's W6
    row-slice (W6 columns host-permuted so output rows are M^T); one bf16
    ReduceScatter sums the partials AND shards the 64 rows 8-per-core.
  - Tail: bias+LeakyReLU+sigmoid+scale into two 128x128 block-diagonal
    bf16 matrices X=diag(M_r^T); the 8-step power iteration becomes
    2 PE squarings (M^4 is converged; X_{k+1}=Y_k^T X_k with
    Y=StreamTranspose(X); scale cancels in the delta ratios); bv is
    extracted straight into an [8, 32] rows-on-partitions layout by two
    accumulated selection matmuls (SEL8A/SEL8B) plus one strided reduce,
    so the delta-coefficient tail runs 8-wide -- no PE transposes, no
    DRAM round trip, no slow single-lane reciprocals.  Final tiny
    AllGather of per-core [4, 32] partial deltas; every core sums the
    8 blocks.
  - Dummy warm-spin matmuls after each layer keep the PE HAM clock at
    2.4 GHz across the collective windows; Lrelu/Sigmoid ACT tables are
    preloaded at startup.
"""

import numpy as np

DIM = 128
N = 32
B = 4
H = 4096
NC = 8          # cores
SL = H // NC    # 512 hidden slice per core
OF = N * N      # 1024 output features
R = 64          # unique MLP rows
HR = 32         # rows per stream
KC = 128        # contraction chunk
SLOPE = 0.01
SC = 64.0       # fp8 weight scale (power of two; exact)
WSPIN = 48      # dummy warm matmuls spanning each AG window

_COMPILED = None
LAST_RESULTS = None


def _build_body(nc, tc, tile, mybir, aps):
    f32 = mybir.dt.float32
    bf16 = mybir.dt.bfloat16
    f8 = mybir.dt.float8e4
    AF = mybir.ActivationFunctionType
    ALU = mybir.AluOpType
    AX = mybir.AxisListType
    rg = [list(range(NC))]

    from contextlib import ExitStack
    es = ExitStack()
    cpool = es.enter_context(tc.tile_pool(name="consts", bufs=1))
    wpool = es.enter_context(tc.tile_pool(name="w", bufs=1))
    bpool = es.enter_context(tc.tile_pool(name="b", bufs=1))
    apool = es.enter_context(tc.tile_pool(name="act", bufs=2))
    atp = es.enter_context(tc.tile_pool(name="atT", bufs=2))
    lpool = es.enter_context(tc.tile_pool(name="lhs", bufs=2))
    tailp = es.enter_context(tc.tile_pool(name="tail", bufs=1))
    ps = es.enter_context(tc.tile_pool(name="ps", bufs=2, space="PSUM"))
    pst = es.enter_context(tc.tile_pool(name="pst", bufs=2, space="PSUM"))
    ps6 = es.enter_context(tc.tile_pool(name="ps6", bufs=1, space="PSUM"))
    tps = es.enter_context(tc.tile_pool(name="tps", bufs=2, space="PSUM"))
    dram = es.enter_context(tc.tile_pool(name="dram", bufs=1, space="DRAM"))

    # ---- preload the Lrelu/Sigmoid activation tables during startup ----
    scr0 = cpool.tile([1, 2], f32)
    nc.vector.memset(scr0[:], 0.0)
    scr1 = cpool.tile([1, 2], f32)
    nc.scalar.activation(scr1[:], scr0[:], AF.Lrelu, alpha=SLOPE)
    scr2 = cpool.tile([1, 2], f32)
    nc.scalar.activation(scr2[:], scr0[:], AF.Sigmoid)

    # ---- all input DMAs up front, in consumption order ----
    id64 = cpool.tile([64, 64], bf16)
    nc.sync.dma_start(id64[:], aps["ID64"][:])
    xt = wpool.tile([KC, 2 * R], bf16, tag="xt")
    nc.sync.dma_start(xt[:], aps["XT"][:])
    wts = {}
    bts = {}
    w1f = wpool.tile([KC, 2 * H], bf16, tag="w1f")
    nc.sync.dma_start(w1f[:], aps["W1F"][:])
    b1f = bpool.tile([KC, H // KC], f32, tag="b1f")
    nc.sync.dma_start(b1f[:], aps["B1F"][:])
    for li in range(2, 7):
        nk = H // KC if li < 6 else 4
        width = OF if li == 6 else SL
        dt = f8 if li <= 5 else bf16
        wts[li] = wpool.tile([KC, nk * width], dt, tag=f"w{li}",
                             name=f"wt{li}")
        nc.sync.dma_start(wts[li][:], aps[f"W{li}"][:])
        if li < 6:
            bts[li] = bpool.tile([1, SL], bf16, tag=f"b{li}", name=f"bt{li}")
            nc.sync.dma_start(bts[li][0:1, :], aps[f"b{li}"].unsqueeze(0))

    # ---- constants (scalar queue; gpsimd stays free for CC doorbells) ----
    bias6 = cpool.tile([128, N], f32)
    nc.scalar.dma_start(bias6[:], aps["BIAS6"][:])
    mac = cpool.tile([128, 2], f32)
    nc.scalar.dma_start(mac[:], aps["MAC"][:])
    dm8 = cpool.tile([8, N], f32)
    nc.scalar.dma_start(dm8[:], aps["DM8"][:])
    tt8 = cpool.tile([8, N], f32)
    nc.scalar.dma_start(tt8[:], aps["TT8"][:])
    w01c = cpool.tile([8, 1], f32)
    nc.scalar.dma_start(w01c[:], aps["W01C"][:])
    sels = cpool.tile([8, B], f32)
    nc.scalar.dma_start(sels[:], aps["SELS"][:])
    selt = cpool.tile([8, B], f32)
    nc.scalar.dma_start(selt[:], aps["SELT"][:])
    sel8a = cpool.tile([128, 8], bf16)
    nc.scalar.dma_start(sel8a[:], aps["SEL8A"][:])
    sel8b = cpool.tile([128, 8], bf16)
    nc.scalar.dma_start(sel8b[:], aps["SEL8B"][:])
    onesb = cpool.tile([1, R], bf16)
    nc.vector.memset(onesb[:], 1.0)
    x1a = tailp.tile([128, 128], bf16, tag="x1a")
    nc.vector.memset(x1a[:], 0.0)
    x1b = tailp.tile([128, 128], bf16, tag="x1b")
    nc.vector.memset(x1b[:], 0.0)

    def layer_mm(chunks, wt, btile, li):
        """chunks: list of (lhs_ap, global_k).  Returns the [128, SL] psum
        with the two 64-wide column-group partial sums in rows [0:64] and
        [64:128]; bias is accumulated into group 0."""
        pt = ps.tile([2 * R, SL], f32, tag="ps", name=f"pt{li}")
        first = [True, True]
        n_h1 = sum(1 for i in range(len(chunks)) if i % 2 == 1)
        seen_h1 = 0
        for i, (lhs, k) in enumerate(chunks):
            h = i % 2
            if h == 1:
                seen_h1 += 1
            nc.tensor.matmul(
                pt[h * R:(h + 1) * R, :],
                lhs,
                wt[:, k * SL:(k + 1) * SL],
                start=first[h],
                stop=(h == 1 and seen_h1 == n_h1),
                tile_position=(0, h * R),
                skip_group_check=True,
            )
            first[h] = False
        nc.tensor.matmul(
            pt[0:R, :], onesb[0:1, :], btile[0:1, :],
            start=first[0], stop=True, tile_position=(0, 0),
            skip_group_check=True,
        )
        return pt

    def act_transpose(pt, li, scale):
        """psum halves -> z -> LeakyReLU (fp8 unscale folded into the
        activation scale) -> bf16 -> transposed fp8 att tile."""
        z0 = apool.tile([R, SL], f32, tag="z0", name=f"z0{li}")
        nc.scalar.activation(z0[:], pt[0:R, :], AF.Copy)
        z = apool.tile([R, SL], f32, tag="z", name=f"z{li}")
        nc.vector.tensor_tensor(z[:], z0[:], pt[R:2 * R, :], op=ALU.add)
        act = apool.tile([R, SL], bf16, tag="act", name=f"act{li}")
        nc.scalar.activation(act[:], z[:], AF.Lrelu, alpha=SLOPE, scale=scale)
        att = atp.tile([KC, 4 * R], f8, tag="att", name=f"att{li}")
        ag_in = dram.tile([KC, 4 * R], f8, tag=f"agin{li}", name=f"agin{li}")
        for j in range(4):
            tp = pst.tile([KC, R], bf16, tag="pst", name=f"tp{li}_{j}")
            nc.tensor.transpose(tp[:], act[:, j * KC:(j + 1) * KC], id64[:])
            nc.vector.tensor_copy(att[:, j * R:(j + 1) * R], tp[:])
            if j % 2 == 1:
                eng = nc.scalar if j == 1 else nc.sync
                eng.dma_start(ag_in[:, (j - 1) * R:(j + 1) * R],
                              att[:, (j - 1) * R:(j + 1) * R])
        return att, ag_in

    def gather(ag_in, li):
        ag_out = dram.tile([NC * KC, 4 * R], f8, tag=f"agout{li}",
                           addr_space="Shared")
        nc.gpsimd.collective_compute(
            "AllGather", ALU.bypass, replica_groups=rg,
            ins=[ag_in[:].opt()], outs=[ag_out[:].opt()],
        )
        engs = [nc.sync, nc.scalar, nc.gpsimd]
        chunks = []
        for r in range(NC):
            lt = lpool.tile([KC, 4 * R], f8, tag=f"lt{r}", name=f"lt{li}_{r}")
            engs[r % 3].dma_start(lt[:], ag_out[r * KC:(r + 1) * KC, :])
            for j in range(4):
                chunks.append((lt[:, j * R:(j + 1) * R], r * 4 + j))
        return chunks

    def warm_spin(count, li, fine=24):
        for i in range(count):
            dpt = ps.tile([2 * R, SL], f32, tag="ps", name=f"wsp{li}_{i}")
            nc.tensor.matmul(dpt[0:R, :], xt[:, 0:R], wts[2][:, 0:SL],
                             start=True, stop=True, tile_position=(0, 0),
                             skip_group_check=True)
        for i in range(fine):
            dpt = ps.tile([2 * R, SL], f32, tag="ps", name=f"wsf{li}_{i}")
            nc.tensor.matmul(dpt[0:R, 0:64], xt[:, 0:R], wts[2][:, 0:64],
                             start=True, stop=True, tile_position=(0, 0),
                             skip_group_check=True)

    # ---- L1: full transposed layer on every core (hidden entirely under
    # the collectives-init barrier; eliminates the first AllGather) ----
    att1 = atp.tile([KC, (H // KC) * R], f8, tag="att1", bufs=1)
    for mt in range(H // KC):
        pc = pst.tile([KC, R], f32, tag="pst", name=f"l1c{mt}")
        nc.tensor.matmul(pc[:], w1f[:, mt * KC:(mt + 1) * KC],
                         xt[:, 0:R], start=True, stop=False,
                         tile_position=(0, 0), skip_group_check=True)
        nc.tensor.matmul(pc[:], w1f[:, H + mt * KC:H + (mt + 1) * KC],
                         xt[:, R:2 * R], start=False, stop=True,
                         tile_position=(0, 0), skip_group_check=True)
        nc.scalar.activation(att1[:, mt * R:(mt + 1) * R], pc[:],
                             AF.Lrelu, alpha=SLOPE,
                             bias=b1f[:, mt:mt + 1])
    chunks = [(att1[:, k * R:(k + 1) * R], k) for k in range(H // KC)]

    # ---- L2..L4 ----
    for li in range(2, 5):
        pt = layer_mm(chunks, wts[li], bts[li], li)
        att, ag_in = act_transpose(pt, li, 1.0 / SC)
        chunks = gather(ag_in, li)
        # L2's block also spans the first-collective premium window
        warm_spin(WSPIN + (56 if li == 2 else 0), li)

    # ---- L5 (no gather) + L6 row-split partial, interleaved ----
    pt5 = ps.tile([2 * R, SL], f32, tag="ps", name="pt5")
    chunks5 = chunks
    pt = layer_mm(chunks5, wts[5], bts[5], 5) if False else None
    # inline L5 matmuls
    first = [True, True]
    seen_h1 = 0
    n_h1 = sum(1 for i in range(len(chunks5)) if i % 2 == 1)
    for i, (lhs, k) in enumerate(chunks5):
        h = i % 2
        if h == 1:
            seen_h1 += 1
        nc.tensor.matmul(pt5[h * R:(h + 1) * R, :], lhs,
                         wts[5][:, k * SL:(k + 1) * SL],
                         start=first[h], stop=(h == 1 and seen_h1 == n_h1),
                         tile_position=(0, h * R), skip_group_check=True)
        first[h] = False
    nc.tensor.matmul(pt5[0:R, :], onesb[0:1, :], bts[5][0:1, :],
                     start=first[0], stop=True, tile_position=(0, 0),
                     skip_group_check=True)
    z05 = apool.tile([R, SL], f32, tag="z0", name="z05")
    nc.scalar.activation(z05[:], pt5[0:R, :], AF.Copy)
    z5 = apool.tile([R, SL], f32, tag="z", name="z5")
    nc.vector.tensor_tensor(z5[:], z05[:], pt5[R:2 * R, :], op=ALU.add)
    act5 = apool.tile([R, SL], bf16, tag="act", name="act5")
    nc.scalar.activation(act5[:], z5[:], AF.Lrelu, alpha=SLOPE, scale=1.0 / SC)
    scr3 = cpool.tile([1, 2], f32)
    nc.scalar.activation(scr3[:], scr0[:], AF.Sigmoid)
    att5 = atp.tile([KC, 4 * R], f8, tag="att", name="att5")
    pt6 = ps6.tile([2 * R, SL], f32, tag="p6")
    for kc in range(4):
        tp = pst.tile([KC, R], bf16, tag="pst", name=f"tp5_{kc}")
        nc.tensor.transpose(tp[:], act5[:, kc * KC:(kc + 1) * KC], id64[:])
        nc.vector.tensor_copy(att5[:, kc * R:(kc + 1) * R], tp[:])
        lhs = att5[:, kc * R:(kc + 1) * R]
        nc.tensor.matmul(pt6[0:R, :], lhs, wts[6][:, kc * OF:kc * OF + SL],
                         start=(kc == 0), stop=(kc == 3),
                         tile_position=(0, 0), skip_group_check=True)
        nc.tensor.matmul(pt6[R:2 * R, :], lhs,
                         wts[6][:, kc * OF + SL:(kc + 1) * OF],
                         start=(kc == 0), stop=(kc == 3),
                         tile_position=(0, R), skip_group_check=True)
    z6 = apool.tile([R, OF], bf16, tag="z6", bufs=1)
    nc.vector.tensor_copy(z6[:, 0:SL], pt6[0:R, :])
    nc.scalar.activation(z6[:, SL:OF], pt6[R:2 * R, :], AF.Copy)
    rs_in = dram.tile([R, OF], bf16, tag="rsin")
    nc.sync.dma_start(rs_in[:], z6[:])
    rs_out = dram.tile([NC, OF], bf16, tag="rsout")
    nc.gpsimd.collective_compute(
        "ReduceScatter", ALU.add, replica_groups=rg,
        ins=[rs_in[:].opt()], outs=[rs_out[:].opt()],
    )

    # ---- tail: 8 rows on this core ----
    zza = tailp.tile([128, N], bf16, tag="zza")
    zzb = tailp.tile([128, N], bf16, tag="zzb")
    nc.sync.dma_start(
        zza[:], rs_out[0:4, :].rearrange("r (j i) -> (r j) i", i=N))
    nc.scalar.dma_start(
        zzb[:], rs_out[4:8, :].rearrange("r (j i) -> (r j) i", i=N))

    def poltile(zz, name, ve):
        zb = tailp.tile([128, N], f32, tag=f"zb_{name}")
        ve.tensor_tensor(zb[:], zz[:], bias6[:], op=ALU.add)
        sc = tailp.tile([128, N], f32, tag=f"sc_{name}")
        ve.tensor_scalar_mul(sc[:], zb[:], SLOPE)
        lr = tailp.tile([128, N], f32, tag=f"lr_{name}")
        ve.tensor_tensor(lr[:], zb[:], sc[:], op=ALU.max)
        sg = tailp.tile([128, N], f32, tag=f"sg_{name}")
        nc.scalar.activation(sg[:], lr[:], AF.Sigmoid)
        pol = tailp.tile([128, N], bf16, tag=f"pol_{name}")
        ve.tensor_scalar(pol[:], sg[:], mac[:, 0:1], mac[:, 1:2],
                         op0=ALU.mult, op1=ALU.add)
        return pol

    pola = poltile(zza, "a", nc.vector)
    polb = poltile(zzb, "b", nc.vector)
    for rl in range(4):
        s = slice(rl * 32, (rl + 1) * 32)
        nc.vector.tensor_copy(x1a[s, s], pola[s, :])
        nc.scalar.activation(x1b[s, s], polb[s, :], AF.Copy)

    def streamT(x, name):
        y = tailp.tile([128, 128], bf16, tag=f"y_{name}")
        nc.vector.transpose(y[:], x[:])
        return y

    y1a = streamT(x1a, "1a")
    y1b = streamT(x1b, "1b")

    def sq(x, y, name, want_y=True):
        px = tps.tile([128, 128], f32, tag="tps", name=f"px{name}")
        nc.tensor.matmul(px[:], y[:], x[:], start=True, stop=True)
        x2 = tailp.tile([128, 128], bf16, tag=f"x_{name}")
        nc.vector.tensor_copy(x2[:], px[:])
        if not want_y:
            return x2, None
        py = tps.tile([128, 128], f32, tag="tps", name=f"py{name}")
        nc.tensor.matmul(py[:], x[:], y[:], start=True, stop=True)
        y2 = tailp.tile([128, 128], bf16, tag=f"y_{name}")
        nc.scalar.activation(y2[:], py[:], AF.Copy)
        return x2, y2

    x2a, y2a = sq(x1a, y1a, "2a")
    x2b, y2b = sq(x1b, y1b, "2b")
    x8a, _ = sq(x2a, y2a, "4a", want_y=False)
    x8b, _ = sq(x2b, y2b, "4b", want_y=False)

    # per-block column sums of X8 via selection matmuls accumulated into
    # one [8, 128] psum (SEL8A maps x8a's 4 blocks to rows 0-3, SEL8B maps
    # x8b's to rows 4-7); one strided reduce densifies to [8, 32].
    bv_ps = tps.tile([8, 128], f32, tag="tps", name="bvps")
    nc.tensor.matmul(bv_ps[:], sel8a[:], x8a[:], start=True, stop=False)
    nc.tensor.matmul(bv_ps[:], sel8b[:], x8b[:], start=False, stop=True)
    bvs = tailp.tile([8, 128], f32, tag="bvs")
    nc.vector.tensor_copy(bvs[:], bv_ps[:])
    bv8 = tailp.tile([8, N], f32, tag="bv8")
    nc.vector.reduce_sum(
        bv8[:], bvs[:].rearrange("p (q j) -> p j q", j=N), axis=AX.X)

    # delta coefficients on 8 partitions
    recipE = tailp.tile([8, N], f32, tag="recipE")
    nc.vector.reciprocal(recipE[:], bv8[:])
    tmp = tailp.tile([8, N], f32, tag="tmp")
    nc.vector.tensor_tensor(tmp[:], bv8[:], dm8[:], op=ALU.mult)
    srcv = tailp.tile([8, 1], f32, tag="srcv")
    nc.vector.reduce_sum(srcv[:], tmp[:], axis=AX.X)
    rd = tailp.tile([8, 1], f32, tag="rd")
    nc.vector.reciprocal(rd[:], srcv[:])
    coefS = tailp.tile([8, 1], f32, tag="coefS")
    nc.vector.tensor_tensor(coefS[:], w01c[:], rd[:], op=ALU.mult)
    tmp2 = tailp.tile([8, N], f32, tag="tmp2")
    nc.vector.tensor_tensor(tmp2[:], tt8[:], recipE[:], op=ALU.mult)
    c23 = tailp.tile([8, 1], f32, tag="c23")
    nc.vector.reduce_sum(c23[:], tmp2[:], axis=AX.X)
    t3 = tailp.tile([8, B], f32, tag="t3")
    nc.vector.tensor_scalar_mul(t3[:], sels[:], coefS[:, 0:1])
    t4 = tailp.tile([8, B], f32, tag="t4")
    nc.vector.tensor_scalar_mul(t4[:], selt[:], c23[:, 0:1])
    coefL = tailp.tile([8, B], f32, tag="coefL")
    nc.vector.tensor_tensor(coefL[:], t3[:], t4[:], op=ALU.add)
    pd_ps = tps.tile([B, N], f32, tag="tps", name="pdps")
    nc.tensor.matmul(pd_ps[:], coefL[:], bv8[:], start=True, stop=True)
    pd = tailp.tile([B, N], f32, tag="pd")
    nc.vector.tensor_copy(pd[:], pd_ps[:])

    # final gather of per-core partial deltas + sum on every core
    agf_in = dram.tile([B, N], f32, tag="agfin")
    nc.scalar.dma_start(agf_in[:], pd[:])
    agf_out = dram.tile([NC * B, N], f32, tag="agfout", addr_space="Shared")
    nc.gpsimd.collective_compute(
        "AllGather", ALU.bypass, replica_groups=rg,
        ins=[agf_in[:].opt()], outs=[agf_out[:].opt()],
    )
    pdall = tailp.tile([B, NC * N], f32, tag="pdall")
    nc.scalar.dma_start(
        pdall[:].rearrange("b (k j) -> b k j", j=N),
        agf_out[:].rearrange("(k b) j -> b k j", b=B),
    )
    osb = tailp.tile([B, N], f32, tag="osb")
    nc.vector.reduce_sum(
        osb[:], pdall[:].rearrange("b (k j) -> b j k", j=N), axis=AX.X)
    nc.sync.dma_start(aps["out"][:], osb[:])
    es.close()


def build():
    import concourse.bacc as bacc
    import concourse.mybir as mybir
    import concourse.tile as tile

    f32 = mybir.dt.float32
    bf16 = mybir.dt.bfloat16
    f8 = mybir.dt.float8e4
    nc = bacc.Bacc("TRN2", target_bir_lowering=False, debug=False, num_devices=NC)
    shapes = {
        "XT": ([KC, 2 * R], bf16),
        "W1F": ([KC, 2 * H], bf16), "B1F": ([KC, H // KC], f32),
        "W2": ([KC, 32 * SL], f8), "b2": ([SL], bf16),
        "W3": ([KC, 32 * SL], f8), "b3": ([SL], bf16),
        "W4": ([KC, 32 * SL], f8), "b4": ([SL], bf16),
        "W5": ([KC, 32 * SL], f8), "b5": ([SL], bf16),
        "W6": ([KC, 4 * OF], bf16),
        "BIAS6": ([128, N], f32), "MAC": ([128, 2], f32),
        "DM8": ([8, N], f32), "TT8": ([8, N], f32), "W01C": ([8, 1], f32),
        "SELS": ([8, B], f32), "SELT": ([8, B], f32),
        "SEL8A": ([128, 8], bf16), "SEL8B": ([128, 8], bf16),
        "ID64": ([64, 64], bf16),
    }
    aps = {
        k: nc.dram_tensor(k, v[0], v[1], kind="ExternalInput").ap()
        for k, v in shapes.items()
    }
    aps["out"] = nc.dram_tensor("out", [B, N], f32, kind="ExternalOutput").ap()
    with tile.TileContext(nc) as tc:
        _build_body(nc, tc, tile, mybir, aps)
    nc.compile()
    return nc


def prep_in_maps(inputs):
    import ml_dtypes
    f = np.float32
    bf = ml_dtypes.bfloat16
    f8 = ml_dtypes.float8_e4m3fn
    E = np.asarray(inputs["batch_node_embeddings"], f)   # (B,N,D)
    T = np.asarray(inputs["batch_Ts"], f)                # (B,N,N)
    mult = np.asarray(inputs["mult_const_batch"], f).reshape(-1)[0]
    add = np.asarray(inputs["add_const_batch"], f).reshape(-1)[0]
    S = np.transpose(E, (1, 0, 2))                       # (N,B,D)
    G0 = np.concatenate([S[:, 0], S[:, 1]], axis=-1)     # (32, 2D)
    G1 = np.concatenate([S[:, 2], S[:, 3]], axis=-1)
    rows = np.concatenate([G0, G1], axis=0)              # (64, 256)

    def packk(Wslice):
        nk = Wslice.shape[0] // KC
        return np.ascontiguousarray(
            Wslice.reshape(nk, KC, -1).transpose(1, 0, 2).reshape(KC, -1)
        )

    perm = np.arange(OF).reshape(N, N).T.reshape(-1)     # perm[j*32+i] = i*32+j
    W6perm = np.asarray(inputs["W6"], f)[:, perm]
    b6p = np.asarray(inputs["b6"], f)[perm]

    common = {
        "XT": packk(rows.T).astype(bf),
        "BIAS6": np.ascontiguousarray(np.tile(b6p.reshape(N, N), (4, 1))),
        "MAC": np.ascontiguousarray(
            np.stack([np.full(128, mult, f), np.full(128, add, f)], axis=1)
        ),
        "ID64": np.eye(64, dtype=bf),
        "SEL8A": np.hstack([
            np.kron(np.eye(4, dtype=f), np.ones((N, 1), f)),
            np.zeros((128, 4), f)]).astype(bf),
        "SEL8B": np.hstack([
            np.zeros((128, 4), f),
            np.kron(np.eye(4, dtype=f), np.ones((N, 1), f))]).astype(bf),
    }
    W1 = np.asarray(inputs["W1"], f)
    b1 = np.asarray(inputs["b1"], f)
    # W1F[p, kc*H + mt*128 + m] = W1[kc*128+p, mt*128+m]
    common["W1F"] = np.ascontiguousarray(
        W1.reshape(2, KC, H).transpose(1, 0, 2).reshape(KC, 2 * H)
    ).astype(bf)
    common["B1F"] = np.ascontiguousarray(
        b1.reshape(H // KC, KC).T.astype(f))
    in_maps = []
    for c in range(NC):
        m = dict(common)
        for li in range(2, 6):
            W = np.asarray(inputs[f"W{li}"], f)
            b = np.asarray(inputs[f"b{li}"], f)
            m[f"W{li}"] = (packk(W[:, c * SL:(c + 1) * SL]) * SC).astype(f8)
            m[f"b{li}"] = np.ascontiguousarray(
                b[c * SL:(c + 1) * SL] * SC).astype(bf)
        m["W6"] = packk(W6perm[c * SL:(c + 1) * SL, :]).astype(bf)
        bS = 0 if c < 4 else 1
        bT = 2 if c < 4 else 3
        dm8 = np.zeros((8, N), f)
        tt8 = np.zeros((8, N), f)
        w01c = np.zeros((8, 1), f)
        sels = np.zeros((8, B), f)
        selt = np.zeros((8, B), f)
        for rl in range(8):
            s = (8 * c + rl) % N
            dm8[rl, s] = 1.0
            tt8[rl] = T[bT][:, s]
            w01c[rl, 0] = T[bS][s, :].sum()
            sels[rl, bS] = 1.0
            selt[rl, bT] = 1.0
        m["DM8"] = dm8
        m["TT8"] = tt8
        m["W01C"] = w01c
        m["SELS"] = sels
        m["SELT"] = selt
        in_maps.append(m)
    return in_maps


def kernel(**inputs):
    global _COMPILED, LAST_RESULTS
    from concourse import bass_utils

    if _COMPILED is None:
        _COMPILED = build()
    in_maps = prep_in_maps(inputs)
    res = bass_utils.run_bass_kernel_spmd(
        _COMPILED, in_maps, core_ids=list(range(NC))
    )
    LAST_RESULTS = res
    return np.asarray(res.results[0]["out"], np.float32)
